# revision 1
# baseline (speedup 1.0000x reference)
"""Causal self-attention (B=8, T=1024, C=1024, H=16, hd=64) on 8 TRN2 cores.

Sharding: data parallel — one batch element per NeuronCore. Each core runs
q/k/v projections + RoPE + causal attention + output projection for its
batch element. All matmuls use float32r (full PE rate at N>=256, ~1.5e-4
rel err vs fp32).

Device layouts (partition dim first):
  xT        [C, T]  x[b].T; moving operand for Q/K proj, stationary for V.
  Q^T, K^T  [d, t]  head-pair hp occupies a [128, T] strip; a per-head
                    feature permutation (evens-then-odds) is folded into
                    the weights so RoPE's q1/q2 split is two contiguous
                    32-partition blocks per 64-row head.
  RoPE: qrot = (q + b) * C2 + (swap32(q) + swap32(b)) * S2m, swap32 done
        with 4 PSUM->SBUF DMAs per tile; C2/S2m are [128, T] cos/sin
        stacks shared by all head pairs.
  S^T   [s, t] per head: lhsT = Krot^T [64, 128] (stationary), rhs =
        Qrot^T [64, <=512]. Head pairs pack into PE row groups (K=64).
        Fully-masked s-blocks are skipped; diagonal blocks compute only
        their valid columns.
  P~ = exp(S^T/8) on ACT straight out of PSUM (float32r out); the
        diagonal 128x128 triangle gets a 0/1 multiply on DVE.
  y^T   [65, t] = [V_j | ones].T @ P~ accumulated over s-tiles j; row 64
        is the softmax denominator r. 1/r is partition-broadcast on
        GPSIMD and multiplied in on DVE while copying to Y^T.
  O^T   [e, t]  out projection of Y^T; host transposes back.
"""
import numpy as np
import concourse.bass as bass
import concourse.tile as tile
import concourse.mybir as mybir
from concourse import bacc
from concourse.bass_utils import run_bass_kernel_spmd

F32 = mybir.dt.float32
F32R = mybir.dt.float32r
EXP = mybir.ActivationFunctionType.Exp
IDENT = mybir.ActivationFunctionType.Identity
ADD = mybir.AluOpType.add
MULT = mybir.AluOpType.mult

B, T, C = 8, 1024, 1024
H, HD = 16, 64
NCORES = 8
TCH = T // 512


def build_program():
    nc = bacc.Bacc("TRN2", target_bir_lowering=False, debug=False)

    def din(name, shape, dt=F32R):
        return nc.dram_tensor(name, shape, dt, kind="ExternalInput").ap()

    xT = din("xT", [C, T])
    wqT = din("wqT", [C, C])
    wkT = din("wkT", [C, C])
    wvT = din("wvT", [C, C])
    woT = din("woT", [C, C])
    bq = din("bq", [128, 8], F32)
    bqs = din("bqs", [128, 8], F32)
    bk = din("bk", [128, 8], F32)
    bks = din("bks", [128, 8], F32)
    bo = din("bo", [128, 8], F32)
    bv = din("bv", [1, C])
    c2 = din("c2", [128, T], F32)
    s2m = din("s2m", [128, T], F32)
    tri = din("tri", [128, 128])
    onesrow = din("onesrow", [1, 128])
    ones16 = din("ones16", [128, 16])
    e2 = din("e2", [2, 128])
    oT = nc.dram_tensor("oT", [C, T], F32, kind="ExternalOutput").ap()

    with tile.TileContext(nc) as tc:
        with (
            tc.tile_pool(name="pc", bufs=1) as pc,
            tc.tile_pool(name="pw", bufs=3) as pw,
            tc.tile_pool(name="pwv", bufs=2) as pwv,
            tc.tile_pool(name="prope", bufs=2) as prope,
            tc.tile_pool(name="ppt", bufs=4) as ppt,
            tc.tile_pool(name="pnorm", bufs=2) as pnorm,
            tc.tile_pool(name="posb", bufs=2) as posb,
            tc.tile_pool(name="psMM", bufs=2, space="PSUM") as psMM,
            tc.tile_pool(name="psY", bufs=2, space="PSUM") as psY,
            tc.tile_pool(name="psS", bufs=2, space="PSUM") as psS,
        ):
            # ---- resident tensors ----
            xT_sb = pc.tile([128, 8 * T], F32R, tag="xbig")
            _dma_engines = [nc.sync, nc.scalar, nc.gpsimd]
            for ct in range(8):
                _dma_engines[ct % 3].dma_start(
                    xT_sb[:, ct * T:(ct + 1) * T],
                    xT[ct * 128:(ct + 1) * 128, :])
            c2_sb = pc.tile([128, T], F32, tag="c2")
            s2_sb = pc.tile([128, T], F32, tag="s2")
            nc.sync.dma_start(c2_sb[:], c2)
            nc.sync.dma_start(s2_sb[:], s2m)
            tri_sb = pc.tile([128, 128], F32R, tag="tri")
            nc.sync.dma_start(tri_sb[:], tri)
            onesrow_sb = pc.tile([1, 128], F32R, tag="onesrow")
            nc.sync.dma_start(onesrow_sb[:], onesrow)
            e2_sb = pc.tile([2, 128], F32R, tag="e2")
            nc.sync.dma_start(e2_sb[:], e2)
            bv_sb = pc.tile([1, C], F32R, tag="bv")
            nc.sync.dma_start(bv_sb[:], bv)
            btiles = {}
            for nm, ap in [("bq", bq), ("bqs", bqs), ("bk", bk), ("bks", bks),
                           ("bo", bo)]:
                t_ = pc.tile([128, 8], F32, tag=nm)
                nc.sync.dma_start(t_[:], ap)
                btiles[nm] = t_
            qrot_sb = pc.tile([128, 8 * T], F32R, tag="qrot")
            krot_sb = pc.tile([128, 8 * T], F32R, tag="krot")
            rpack = pc.tile([64, 512], F32R, tag="rpack")
            # V per s-tile: [128, 16 heads x (64 cols + ones col)]
            v_sb = [pc.tile([128, 16 * 65], F32R, tag=f"v{j}", name=f"v{j}")
                    for j in range(8)]
            v3 = [v_sb[j][:].rearrange("p (h j) -> p h j", j=65) for j in range(8)]
            for j in range(8):
                nc.sync.dma_start(v3[j][:, :, 64:65], ones16)

            # ---- emission helpers (projections become PE filler work
            # inside the attention loop so the PE never idles/re-throttles) --
            qk_slabs = {}

            def emit_qk_group(which, dblk, ch):
                wT, bnm, bsnm, dest = which
                key = (bnm, dblk)
                if key not in qk_slabs:
                    wsl = pw.tile([128, 8, 128], F32R, tag="w",
                                  name=f"w{bnm}{dblk}")
                    nc.sync.dma_start(
                        wsl[:],
                        wT[:, dblk * 128:(dblk + 1) * 128].rearrange(
                            "(ct p) m -> p ct m", p=128))
                    qk_slabs[key] = wsl
                wsl = qk_slabs[key]
                ps = psMM.tile([128, 512], F32, tag="mm",
                               name=f"p{bnm}{dblk}_{ch}")
                for ct in range(8):
                    nc.tensor.matmul(
                        ps[:], wsl[:, ct, :],
                        xT_sb[:, ct * T + ch * 512: ct * T + ch * 512 + 512],
                        start=(ct == 0), stop=(ct == 7))
                qsw = prope.tile([128, 512], F32, tag="qsw",
                                 name=f"qsw{bnm}{dblk}_{ch}")
                nc.vector.stream_shuffle(
                    qsw[:], ps[:],
                    mask=list(range(16, 32)) + list(range(0, 16)))
                dsl = dest[:, dblk * T + ch * 512: dblk * T + ch * 512 + 512]
                nc.vector.scalar_tensor_tensor(
                    dsl, ps[:], btiles[bnm][:, dblk:dblk + 1],
                    c2_sb[:, ch * 512:ch * 512 + 512], op0=ADD, op1=MULT)
                nc.vector.scalar_tensor_tensor(
                    qsw[:], qsw[:], btiles[bsnm][:, dblk:dblk + 1],
                    s2_sb[:, ch * 512:ch * 512 + 512], op0=ADD, op1=MULT)
                nc.gpsimd.tensor_add(dsl, dsl, qsw[:])

            wv_slabs = {}

            def emit_v_group(ch, sblk):
                if (ch, 0) not in wv_slabs:
                    wv_r = wvT[:, ch * 512:(ch + 1) * 512].rearrange(
                        "(ct p) m -> p ct m", p=128)
                    for half in range(2):
                        vsl = pwv.tile([128, 4, 512], F32R, tag="wv",
                                       name=f"wv{ch}_{half}")
                        nc.sync.dma_start(vsl[:],
                                          wv_r[:, half * 4:half * 4 + 4, :])
                        wv_slabs[(ch, half)] = vsl
                ps = psMM.tile([128, 512], F32, tag="mm", name=f"pv{ch}_{sblk}")
                for ct in range(8):
                    vsl = wv_slabs[(ch, ct // 4)]
                    nc.tensor.matmul(
                        ps[:],
                        xT_sb[:, ct * T + sblk * 128: ct * T + sblk * 128 + 128],
                        vsl[:, ct % 4, :],
                        start=(ct == 0), stop=False)
                nc.tensor.matmul(
                    ps[:], onesrow_sb[:], bv_sb[:, ch * 512:(ch + 1) * 512],
                    start=False, stop=True)
                nc.vector.tensor_copy(v3[sblk][:, 8 * ch:8 * ch + 8, 0:64],
                                      ps[:])

            QSPEC = (wqT, "bq", "bqs", qrot_sb)
            KSPEC = (wkT, "bk", "bks", krot_sb)

            # upfront: pair 0's Q/K strips + V chunk 0 (heads 0-7)
            for ch in range(TCH):
                emit_qk_group(QSPEC, 0, ch)
            for ch in range(TCH):
                emit_qk_group(KSPEC, 0, ch)
            for sblk in range(8):
                emit_v_group(0, sblk)

            # ---- attention with interleaved projection filler ----
            yt_sb = pc.tile([128, 8 * T], F32R, tag="xbig")  # reuses xT slot
            for hp in range(8):
                fillers = []
                if hp < 7:
                    for ch in range(TCH):
                        fillers.append(lambda d=hp + 1, c=ch:
                                       emit_qk_group(QSPEC, d, c))
                    for ch in range(TCH):
                        fillers.append(lambda d=hp + 1, c=ch:
                                       emit_qk_group(KSPEC, d, c))
                if hp < 3:
                    lo, hi_ = {0: (0, 3), 1: (3, 6), 2: (6, 8)}[hp]
                    for sblk in range(lo, hi_):
                        fillers.append(lambda s=sblk: emit_v_group(1, s))
                base = hp * T
                for c in range(TCH):
                    njs = 4 * c + 4
                    ps_y = [psY.tile([65, 512], F32, tag="y",
                                      name=f"y{hp}_{c}_{k}")
                            for k in range(2)]
                    nsts = [128 * (j - 4 * c) if j >= 4 * c else 0
                            for j in range(njs)]
                    p_all = []
                    for j in range(njs):
                        nst = nsts[j]
                        # both heads' S^T side by side in one 2-bank psum
                        ps_s = psS.tile([128, 1024], F32, tag="s")
                        ps_s3 = ps_s[:].rearrange("p (g t) -> p g t", g=2)
                        for hi in range(2):
                            r0 = 64 * hi
                            nc.tensor.matmul(
                                ps_s3[:, hi, nst:512],
                                krot_sb[r0:r0 + 64,
                                        base + j * 128: base + j * 128 + 128],
                                qrot_sb[r0:r0 + 64,
                                        base + c * 512 + nst: base + c * 512 + 512],
                                start=True, stop=True)
                        p_t = ppt.tile([128, 1024], F32R, tag="pt")
                        p_t3 = p_t[:].rearrange("p (g t) -> p g t", g=2)
                        nc.scalar.activation(p_t3[:, :, nst:512],
                                             ps_s3[:, :, nst:512],
                                             EXP, scale=0.125)
                        if j >= 4 * c:
                            for hi in range(2):
                                nc.vector.tensor_mul(
                                    p_t3[:, hi, nst:nst + 128],
                                    p_t3[:, hi, nst:nst + 128],
                                    tri_sb[:])
                        p_all.append(p_t3)
                        # PE filler keeps the array busy across the exp/mask
                        # latency so HAM stays at K=8/8
                        if fillers:
                            fillers.pop(0)()
                        if j >= 2:
                            jj = j - 2
                            for hi in range(2):
                                h = 2 * hp + hi
                                nc.tensor.matmul(
                                    ps_y[hi][:, nsts[jj]:512],
                                    v_sb[jj][:, 65 * h: 65 * h + 65],
                                    p_all[jj][:, hi, nsts[jj]:512],
                                    start=(jj == 0), stop=False)
                    for f in fillers:
                        f()
                    fillers = []
                    for jj in range(max(0, njs - 2), njs):
                        for hi in range(2):
                            h = 2 * hp + hi
                            nc.tensor.matmul(
                                ps_y[hi][:, nsts[jj]:512],
                                v_sb[jj][:, 65 * h: 65 * h + 65],
                                p_all[jj][:, hi, nsts[jj]:512],
                                start=(jj == 0), stop=(jj == njs - 1))
                    # stash unnormalized y and the row sums; normalize later
                    for hi in range(2):
                        h = 2 * hp + hi
                        rr = c * 32 + h
                        rtmp = pnorm.tile([1, 512], F32R, tag="rtmp")
                        nc.vector.tensor_copy(rtmp[:], ps_y[hi][64:65, :])
                        nc.sync.dma_start(rpack[rr:rr + 1, :], rtmp[:])
                        nc.vector.tensor_copy(
                            yt_sb[64 * hi:64 * hi + 64,
                                  base + c * 512: base + c * 512 + 512],
                            ps_y[hi][0:64, :])

            # ---- deferred softmax normalization ----
            # 1/r for all heads in one shot, then a K=2 PE matmul broadcasts
            # each head pair's reciprocals into a [128, 512] PSUM tile
            for c in range(TCH):
                with nc.allow_low_precision(reason="float32r reciprocal feed"):
                    nc.vector.reciprocal(rpack[c * 32:c * 32 + 32, :],
                                         rpack[c * 32:c * 32 + 32, :])
                for hp in range(8):
                    base = hp * T
                    rinv2 = pnorm.tile([2, 512], F32R, tag="rinv2")
                    for hi in range(2):
                        rr = c * 32 + 2 * hp + hi
                        nc.sync.dma_start(rinv2[hi:hi + 1, :],
                                          rpack[rr:rr + 1, :])
                    rbc = psMM.tile([128, 512], F32, tag="mm",
                                    name=f"rbc{hp}_{c}")
                    nc.tensor.matmul(rbc[:], e2_sb[:], rinv2[:],
                                     start=True, stop=True)
                    ysl = yt_sb[:, base + c * 512: base + c * 512 + 512]
                    nc.vector.tensor_mul(ysl, ysl, rbc[:])

            # ---- output projection ----
            for eblk in range(8):
                wsl = pw.tile([128, 8, 128], F32R, tag="w")
                nc.sync.dma_start(
                    wsl[:],
                    woT[:, eblk * 128:(eblk + 1) * 128].rearrange(
                        "(ct p) m -> p ct m", p=128),
                )
                for ch in range(TCH):
                    ps = psMM.tile([128, 512], F32, tag="mm")
                    for dt in range(8):
                        nc.tensor.matmul(
                            ps[:],
                            wsl[:, dt, :],
                            yt_sb[:, dt * T + ch * 512: dt * T + ch * 512 + 512],
                            start=(dt == 0), stop=(dt == 7),
                        )
                    osb = posb.tile([128, 512], F32, tag="osb")
                    nc.vector.tensor_scalar_add(osb[:], ps[:],
                                                btiles["bo"][:, eblk:eblk + 1])
                    nc.sync.dma_start(
                        oT[eblk * 128:(eblk + 1) * 128, ch * 512:(ch + 1) * 512],
                        osb[:])

    nc.compile()
    return nc


def prep_inputs(x, wq, bq, wk, bk, wv, bv, wo, bo):
    """Host-side prep: per-head feature permutation, transposes, RoPE tables."""
    f32 = np.float32
    # interleave-16 feature order per head: [q1[0:16], q2[0:16],
    # q1[16:32], q2[16:32]] where q1 = even orig features, q2 = odd.
    perm = np.concatenate([
        np.arange(0, 32, 2), np.arange(1, 32, 2),
        np.arange(32, 64, 2), np.arange(33, 64, 2),
    ])
    pidx = np.concatenate([h * HD + perm for h in range(H)])

    wq_p, bq_p = wq[pidx], bq[pidx]
    wk_p, bk_p = wk[pidx], bk[pidx]
    # swap the 16-row halves within every 32-row quadrant
    swap = lambda v: np.ascontiguousarray(
        v.reshape(2 * H, 2, 16)[:, ::-1].reshape(-1))
    bt = lambda v: np.ascontiguousarray(v.reshape(8, 128).T, dtype=f32)

    inv_freq = (1.0 / (10000.0 ** (np.arange(0, HD, 2, dtype=np.float64) / HD)))
    th = np.outer(np.arange(T, dtype=np.float64), inv_freq)  # [T, 32]
    cosT = np.cos(th).T.astype(f32)  # [32, T]
    sinT = np.sin(th).T.astype(f32)
    c64 = np.concatenate([cosT[0:16], cosT[0:16], cosT[16:32], cosT[16:32]])
    s64 = np.concatenate([-sinT[0:16], sinT[0:16], -sinT[16:32], sinT[16:32]])
    c2 = np.ascontiguousarray(np.tile(c64, (2, 1)))  # [128, T]
    s2m = np.ascontiguousarray(np.tile(s64, (2, 1)))

    shared = {
        "wqT": np.ascontiguousarray(wq_p.T, dtype=f32),
        "wkT": np.ascontiguousarray(wk_p.T, dtype=f32),
        "wvT": np.ascontiguousarray(wv.T, dtype=f32),
        "woT": np.ascontiguousarray(wo.T, dtype=f32),
        "bq": bt(bq_p), "bqs": bt(swap(bq_p)),
        "bk": bt(bk_p), "bks": bt(swap(bk_p)),
        "bo": bt(bo),
        "bv": np.ascontiguousarray(bv[None, :], dtype=f32),
        "c2": c2, "s2m": s2m,
        "tri": np.triu(np.ones((128, 128), dtype=f32)),
        "onesrow": np.ones((1, 128), dtype=f32),
        "e2": np.repeat(np.eye(2, dtype=f32), 64, axis=1),
        "ones16": np.ones((128, 16), dtype=f32),
    }
    in_maps = []
    for b in range(B):
        m = dict(shared)
        m["xT"] = np.ascontiguousarray(np.asarray(x[b]).T, dtype=f32)
        in_maps.append(m)
    return in_maps


_nc_cache = None


def run(inputs, trace=False, trace_kwargs=None):
    global _nc_cache
    if _nc_cache is None:
        _nc_cache = build_program()
    in_maps = prep_inputs(
        np.asarray(inputs["x"], dtype=np.float32),
        *[np.asarray(inputs[k], dtype=np.float32)
          for k in ["wq", "bq", "wk", "bk", "wv", "bv", "wo", "bo"]])
    res = run_bass_kernel_spmd(_nc_cache, in_maps, list(range(NCORES)),
                               trace=trace, **(trace_kwargs or {}))
    out = np.stack([np.ascontiguousarray(res.results[b]["oT"].T)
                    for b in range(B)]).astype(np.float32)
    return out, res


def kernel(**inputs):
    out, _ = run(inputs, trace=False)
    return out

